# revision 10
# baseline (speedup 1.0000x reference)
"""Trainium2 Bass kernel for nn_PraxisScatter (moe_routing).

Strategy (8 NeuronCores, tensor-parallel over the hidden dim H=4096):
  - Each core owns a 512-row shard of H for gate1/gate2/up/down weights.
  - gate1: gT = relu(w1 @ xT + b1) on the shard, 3-term bf16 hi/lo matmuls
    (the top-k selection needs exact-grade scores).
  - g is AllGathered as six chunks: hi parts first (gate2's first sweep
    starts as soon as hi chunks land), lo parts behind them.
  - gate2 sweep 1 accumulates the two g_hi terms (w_hi+w_lo vs g_hi);
    the partial scores are AllToAll'd and the threshold search runs on
    them (sigma-estimate bracket + 9 Illinois rounds) WHILE sweep 2
    (w_hi vs g_lo) finishes the exact scores on TensorE.
  - Exact scores are AllToAll'd; 8 more Illinois rounds in a +-4e-4
    bracket around the partial threshold land in the exact count==K
    plateau (counts split: vector engine is_gt half, scalar engine Sign
    half, in parallel; partition reduce on GpSimd).
  - main: h_prev/h_curr = up @ xT as single-term bf16 matmuls (the value
    path only needs ~1e-2 accuracy), per-(h,batch) select in bf16,
    exact Gelu on ScalarE.
  - down: single-term bf16 partials; bf16 ReduceScatter in four
    D-quarters so collectives overlap the matmuls and only the last
    quarter's transfer is tail-exposed.
"""

import sys

try:
    import concourse  # noqa: F401
except ImportError:  # pragma: no cover
    sys.path.insert(0, "/opt/trn_rl_repo")

import contextlib

import ml_dtypes
import numpy as np

import concourse.bass as bass  # noqa: F401
import concourse.bass_isa as bass_isa
import concourse.mybir as mybir
import concourse.tile as tile
from concourse import bacc
from concourse.bass_utils import run_bass_kernel_spmd

BF16 = ml_dtypes.bfloat16
F32 = np.float32

NCORES = 8
B, S, D, H = 8, 128, 1024, 4096
T = B * S              # 1024 tokens
HS = H // NCORES       # 512 h rows per core
K_SEL = 256 * S        # 32768
QNORM = 1.5341         # Phi^-1(1 - K_SEL/(S*H))
BRK = 0.04             # part-1 bracket half-width around sigma estimate
BRK2 = 4e-4            # part-2 bracket half-width around partial threshold
R_ITER1 = 9            # Illinois rounds on partial scores (hidden)
R_ITER2 = 8            # Illinois rounds on exact scores

f32 = mybir.dt.float32
bf16 = mybir.dt.bfloat16
AF = mybir.ActivationFunctionType
OP = mybir.AluOpType


def _build():
    nc = bacc.Bacc("TRN2", target_bir_lowering=False, debug=False,
                   num_devices=NCORES)

    # ---- per-core DRAM parameters (host pre-transposed, partition-major) ----
    xhl_d = nc.dram_tensor("xhl", [128, 8, 2, T], bf16, kind="ExternalInput").ap()
    w1_d = nc.dram_tensor("w1hl", [128, 8, 2, HS], bf16, kind="ExternalInput").ap()
    w2_d = nc.dram_tensor("w2hl", [H, 2, HS], bf16, kind="ExternalInput").ap()
    w2h2_d = nc.dram_tensor("w2h2", [H, HS], bf16, kind="ExternalInput").ap()
    up_p_d = nc.dram_tensor("uppb", [128, 8, HS], bf16, kind="ExternalInput").ap()
    up_c_d = nc.dram_tensor("upcb", [128, 8, HS], bf16, kind="ExternalInput").ap()
    dw_d = nc.dram_tensor("dwb", [128, 4, D], bf16, kind="ExternalInput").ap()
    b1_d = nc.dram_tensor("b1s", [128, 4], f32, kind="ExternalInput").ap()
    b2_d = nc.dram_tensor("b2s", [128, 4], f32, kind="ExternalInput").ap()
    bp_d = nc.dram_tensor("bps", [128, 4], f32, kind="ExternalInput").ap()
    bc_d = nc.dram_tensor("bcs", [128, 4], f32, kind="ExternalInput").ap()
    dbias_d = nc.dram_tensor("dbias", [128, D], f32, kind="ExternalInput").ap()
    out_d = nc.dram_tensor("out", [S, D], f32, kind="ExternalOutput").ap()

    # ---- internal DRAM (collective bounce buffers) ----
    wu_in = nc.dram_tensor("wu_in", [8], f32).ap()
    wu_out = nc.dram_tensor("wu_out", [64], f32, addr_space="Shared").ap()
    wu2_out = nc.dram_tensor("wu2_out", [8], f32).ap()
    wu_rs_in = nc.dram_tensor("wu_rs_in", [64], f32).ap()
    wu3_out = nc.dram_tensor("wu3_out", [8], f32).ap()
    _ag_rows = [128, 128, 256]
    g_ag_in = {(0, j): nc.dram_tensor(f"g_ag_in_0{j}", [_ag_rows[j], T],
                                      bf16).ap()
               for j in range(3)}
    g_ag_out = {(0, j): nc.dram_tensor(f"g_ag_out_0{j}",
                                       [NCORES * _ag_rows[j], T], bf16,
                                       addr_space="Shared").ap()
                for j in range(3)}
    g_lo_in = nc.dram_tensor("g_lo_in", [HS, T], bf16).ap()
    g_lo_out = nc.dram_tensor("g_lo_out", [H, T], bf16,
                              addr_space="Shared").ap()
    a2a_in = [nc.dram_tensor(f"a2a_in{i}", [NCORES, HS, S], f32).ap()
              for i in range(2)]
    a2a_out = [nc.dram_tensor(f"a2a_out{i}", [NCORES, HS, S], f32).ap()
               for i in range(2)]
    t_ag_in = nc.dram_tensor("t_ag_in", [8], f32).ap()
    t_ag_out = nc.dram_tensor("t_ag_out", [64], f32, addr_space="Shared").ap()
    rs_in = [nc.dram_tensor(f"rs_in{q}", [B, S, 512], bf16).ap() for q in range(2)]
    rs_out = [nc.dram_tensor(f"rs_out{q}", [S, 512], bf16).ap() for q in range(2)]

    rg = [list(range(NCORES))]

    with tile.TileContext(nc) as tc, contextlib.ExitStack() as ctx:
        en = tc.nc
        const = ctx.enter_context(tc.tile_pool(name="const", bufs=1))
        xp = ctx.enter_context(tc.tile_pool(name="xres", bufs=1))
        wks = ctx.enter_context(tc.tile_pool(name="wks", bufs=5))
        gkp = ctx.enter_context(tc.tile_pool(name="gkp", bufs=3))
        gact = ctx.enter_context(tc.tile_pool(name="gact", bufs=1))
        big = ctx.enter_context(tc.tile_pool(name="big", bufs=1))
        scp = ctx.enter_context(tc.tile_pool(name="scp", bufs=2))
        outp = ctx.enter_context(tc.tile_pool(name="outp", bufs=2))
        ps = ctx.enter_context(tc.tile_pool(name="ps", bufs=8, space="PSUM"))

        _cc_prev = [None]

        def cc(kind, op, ins, outs, waits=()):
            """Issue a collective: explicitly depend on every DMA that wrote
            the input buffer, and chain collectives so every core issues
            them in one fixed order."""
            h = en.gpsimd.collective_compute(kind, op, ins=ins, outs=outs,
                                             replica_groups=rg)
            for w in waits:
                tile.add_dep_helper(h.ins, w.ins,
                                    reason="collective input writer")
            if _cc_prev[0] is not None:
                tile.add_dep_helper(h.ins, _cc_prev[0].ins,
                                    reason="collective issue-order chain")
            _cc_prev[0] = h
            return h

        # warmup: absorb the CC engine's first-collective startup cost early
        cc("AllGather", OP.bypass, [wu_in[:]], [wu_out[:]])
        cc("AllToAll", OP.bypass, [wu_in[:]], [wu2_out[:]])
        cc("ReduceScatter", OP.add, [wu_rs_in[:]], [wu3_out[:]])

        def mm3_pair(p0, p1, lhsT_tile, rhs_tile, mslc, first, last):
            """3-term hi/lo accumulation into the (n0, n1) psum pair, ordered
            so consecutive matmuls share the stationary operand."""
            w_hi, w_lo = lhsT_tile[:, 0, mslc], lhsT_tile[:, 1, mslc]
            n0, n1 = slice(0, 512), slice(512, 1024)
            en.tensor.matmul(p0[:], w_hi, rhs_tile[:, 0, n0], start=first, stop=False)
            en.tensor.matmul(p1[:], w_hi, rhs_tile[:, 0, n1], start=first, stop=False)
            en.tensor.matmul(p0[:], w_hi, rhs_tile[:, 1, n0], start=False, stop=False)
            en.tensor.matmul(p1[:], w_hi, rhs_tile[:, 1, n1], start=False, stop=False)
            en.tensor.matmul(p0[:], w_lo, rhs_tile[:, 0, n0], start=False, stop=last)
            en.tensor.matmul(p1[:], w_lo, rhs_tile[:, 0, n1], start=False, stop=last)

        # ---------- resident loads (chunked so gate1 starts early) ----------
        x_s = xp.tile([128, 8, 2, T], bf16, tag="x")
        w1r = xp.tile([128, 8, 2, HS], bf16, tag="w1r")
        for k in range(8):
            en.sync.dma_start(x_s[:, k], xhl_d[:, k])
            en.sync.dma_start(w1r[:, k], w1_d[:, k])
        b1_s = const.tile([128, 4], f32, tag="b1")
        en.sync.dma_start(b1_s[:], b1_d[:])
        b2_s = const.tile([128, 4], f32, tag="b2")
        en.sync.dma_start(b2_s[:], b2_d[:])
        bp_s = const.tile([128, 4], f32, tag="bp")
        en.sync.dma_start(bp_s[:], bp_d[:])
        bc_s = const.tile([128, 4], f32, tag="bc")
        en.sync.dma_start(bc_s[:], bc_d[:])

        # ---------- gate1: gT = relu(w1s @ xT + b1) [HS, T] ----------
        # m-tiles complete one at a time (k inner); each hi AllGather chunk
        # fires while the next m-tile is computing, lo chunks follow.
        g_sb = gact.tile([128, 4, 2, T], bf16, tag="gact", name="g_sb")
        hi_waits, lo_waits = [], []
        for m in range(4):
            mslc = slice(m * 128, (m + 1) * 128)
            pts = {n: ps.tile([128, 512], f32, tag="ps", name=f"g1_{m}_{n}")
                   for n in range(2)}
            for k in range(8):
                mm3_pair(pts[0], pts[1], w1r[:, k], x_s[:, k], mslc,
                         first=(k == 0), last=(k == 7))
            gf = big.tile([128, T], f32, tag="gf", name=f"gf{m}")
            for n in range(2):
                nslc = slice(n * 512, (n + 1) * 512)
                en.scalar.activation(gf[:, nslc], pts[n][:],
                                     AF.Relu, bias=b1_s[:, m:m + 1])
            # split to bf16 hi/lo
            en.vector.tensor_copy(g_sb[:, m, 0, :], gf[:])
            en.vector.tensor_sub(g_sb[:, m, 1, :], gf[:], g_sb[:, m, 0, :])
            j = m if m < 2 else 2
            u = 0 if m < 2 else m - 2
            hi_waits.append(en.sync.dma_start(
                g_ag_in[(0, j)][u * 128:(u + 1) * 128], g_sb[:, m, 0]))
            lo_waits.append(en.sync.dma_start(
                g_lo_in[m * 128:(m + 1) * 128], g_sb[:, m, 1]))
            if m != 2:
                cc("AllGather", OP.bypass, [g_ag_in[(0, j)][:]],
                   [g_ag_out[(0, j)][:]], waits=tuple(hi_waits))
                hi_waits = []
        cc("AllGather", OP.bypass, [g_lo_in[:]], [g_lo_out[:]],
           waits=tuple(lo_waits))

        # ---------- gate2 sweep 1: (w_hi + w_lo) vs g_hi ----------
        # K-tile kt order follows the 3 AllGather chunks; the host permutes
        # w2 rows to match.
        def kt_src(p, kt):
            if p == 1:
                return g_lo_out[kt * 128:(kt + 1) * 128]
            if kt < 8:
                return g_ag_out[(p, 0)][kt * 128:(kt + 1) * 128]
            if kt < 16:
                return g_ag_out[(p, 1)][(kt - 8) * 128:(kt - 7) * 128]
            idx = kt - 16
            rr, u = idx // 2, idx % 2
            return g_ag_out[(p, 2)][rr * 256 + u * 128:rr * 256 + (u + 1) * 128]

        g2ps = {(m, n): ps.tile([128, 512], f32, tag="ps", name=f"g2_{m}_{n}")
                for m in range(4) for n in range(2)}
        n0, n1 = slice(0, 512), slice(512, 1024)
        for kt in range(32):
            w2k = wks.tile([128, 2, HS], bf16, tag="wk", name=f"w2k{kt}")
            en.sync.dma_start(w2k[:], w2_d[kt * 128:(kt + 1) * 128])
            gk = gkp.tile([128, T], bf16, tag="gkh", name=f"gkh{kt}")
            en.sync.dma_start(gk[:], kt_src(0, kt))
            for m in range(4):
                mslc = slice(m * 128, (m + 1) * 128)
                w_hi, w_lo = w2k[:, 0, mslc], w2k[:, 1, mslc]
                p0, p1 = g2ps[(m, 0)], g2ps[(m, 1)]
                first = kt == 0
                en.tensor.matmul(p0[:], w_hi, gk[:, n0], start=first, stop=False)
                en.tensor.matmul(p1[:], w_hi, gk[:, n1], start=first, stop=False)
                en.tensor.matmul(p0[:], w_lo, gk[:, n0], start=False,
                                 stop=(kt == 31))
                en.tensor.matmul(p1[:], w_lo, gk[:, n1], start=False,
                                 stop=(kt == 31))

        # partial scores -> A2A #1 (feeds the hidden threshold search)
        a2a1_writers = []
        for m in range(4):
            scm = scp.tile([128, T], f32, tag="scp", name=f"scp{m}")
            for n in range(2):
                nslc = slice(n * 512, (n + 1) * 512)
                en.scalar.activation(scm[:, nslc], g2ps[(m, n)][:],
                                     AF.Identity, bias=b2_s[:, m:m + 1])
            a2a1_writers.append(en.sync.dma_start(
                a2a_in[0][:, m * 128:(m + 1) * 128, :].rearrange("j p t -> p j t"),
                scm[:]))
        cc("AllToAll", OP.bypass, [a2a_in[0][:]], [a2a_out[0][:]],
           waits=tuple(a2a1_writers))

        scb = big.tile([128, H], f32, tag="scb", name="scb")
        fill1 = en.sync.dma_start(
            scb[:], a2a_out[0].rearrange("r h t -> (r h t)").rearrange(
                "(p f) -> p f", p=128))
        tile.add_dep_helper(fill1.ins, _cc_prev[0].ins, reason="a2a1 -> fill")
        cmpb = big.tile([128, H], bf16, tag="cmpb", name="cmpb")
        up_s = {0: xp.tile([128, 8, HS], bf16, tag="upp", name="up_s0"),
                1: xp.tile([128, 8, HS], bf16, tag="upc", name="up_s1")}
        for u, src_d in ((0, up_p_d), (1, up_c_d)):
            en.sync.dma_start(up_s[u][:], src_d[:])
        dw_s = xp.tile([128, 4, D], bf16, tag="dw")
        en.sync.dma_start(dw_s[:], dw_d[:])
        dbias_s = const.tile([128, D], f32, tag="dbias")
        en.sync.dma_start(dbias_s[:], dbias_d[:])

        def sv(tag):
            return const.tile([128, 1], f32, tag=tag, name=tag)

        lo, hi, fl, fh = sv("lo"), sv("hi"), sv("fl"), sv("fh")
        tt, acc, acs, cnt = sv("tt"), sv("acc"), sv("acs"), sv("cnt")
        s1, s2, s3 = sv("s1"), sv("s2"), sv("s3")
        gt, ng, sig = sv("gt"), sv("ng"), sv("sig")
        HALF = H // 2
        CNT_OFF = float(K_SEL) - 0.5 - 128 * (HALF // 2)

        def count_pass(t_tile, f_out):
            """f_out = count(scb > t) - (K_SEL - 0.5), replicated on lanes.

            Vector engine counts the first half (is_gt), scalar engine the
            second half (Sign(t - s) summed), in parallel.
            """
            en.vector.tensor_scalar(cmpb[:, :HALF], scb[:, :HALF], t_tile[:],
                                    0.0, op0=OP.is_gt, op1=OP.add,
                                    accum_out=acc[:])
            en.scalar.activation(cmpb[:, HALF:], scb[:, HALF:], AF.Sign,
                                 bias=t_tile[:], scale=-1.0, accum_out=acs[:])
            en.vector.scalar_tensor_tensor(s3[:], acs[:], -0.5, acc[:],
                                           op0=OP.mult, op1=OP.add)
            en.gpsimd.partition_all_reduce(cnt[:], s3[:], channels=128,
                                           reduce_op=bass_isa.ReduceOp.add)
            en.vector.tensor_scalar(f_out[:], cnt[:], CNT_OFF, None,
                                    op0=OP.subtract)

        def illinois_rounds(n_rounds):
            for r in range(n_rounds):
                if (r + 1) % 4 == 0:
                    # midpoint fallback round (guards against secant stalls)
                    en.vector.tensor_tensor(s2[:], lo[:], hi[:], op=OP.add)
                    en.vector.tensor_scalar(tt[:], s2[:], 0.5, None, op0=OP.mult)
                else:
                    # frac = clip(fl/(fl-fh), .02, .98); t = lo + frac*(hi-lo)
                    en.vector.tensor_tensor(s1[:], fl[:], fh[:], op=OP.subtract)
                    en.vector.reciprocal(s2[:], s1[:])
                    en.vector.tensor_tensor(s3[:], fl[:], s2[:], op=OP.mult)
                    en.vector.tensor_scalar(s3[:], s3[:], 0.02, 0.98,
                                            op0=OP.max, op1=OP.min)
                    en.vector.tensor_tensor(s1[:], hi[:], lo[:], op=OP.subtract)
                    en.vector.scalar_tensor_tensor(tt[:], s1[:], s3[:], lo[:],
                                                   op0=OP.mult, op1=OP.add)
                count_pass(tt, s1)  # s1 = f_t
                en.vector.tensor_scalar(gt[:], s1[:], 0.0, None, op0=OP.is_ge)
                en.vector.tensor_scalar(ng[:], gt[:], 1.0, -1.0,
                                        op0=OP.subtract, op1=OP.mult)  # 1-gt
                en.vector.tensor_tensor(s2[:], tt[:], lo[:], op=OP.subtract)
                en.vector.scalar_tensor_tensor(lo[:], s2[:], gt[:], lo[:],
                                               op0=OP.mult, op1=OP.add)
                en.vector.tensor_tensor(s2[:], s1[:], fl[:], op=OP.subtract)
                en.vector.scalar_tensor_tensor(fl[:], s2[:], gt[:], fl[:],
                                               op0=OP.mult, op1=OP.add)
                en.vector.tensor_tensor(s2[:], tt[:], hi[:], op=OP.subtract)
                en.vector.scalar_tensor_tensor(hi[:], s2[:], ng[:], hi[:],
                                               op0=OP.mult, op1=OP.add)
                en.vector.tensor_tensor(s2[:], s1[:], fh[:], op=OP.subtract)
                en.vector.scalar_tensor_tensor(fh[:], s2[:], ng[:], fh[:],
                                               op0=OP.mult, op1=OP.add)

        # part 1 (on partial scores; hidden under gate2 sweep 2)
        ssq = en.scalar.activation(cmpb[:], scb[:], AF.Square, accum_out=acc[:])
        tile.add_dep_helper(ssq.ins, fill1.ins, reason="scb fill barrier")
        en.gpsimd.partition_all_reduce(cnt[:], acc[:], channels=128,
                                       reduce_op=bass_isa.ReduceOp.add)
        en.scalar.activation(sig[:], cnt[:], AF.Sqrt, scale=1.0 / (S * H))
        en.vector.tensor_scalar(lo[:], sig[:], QNORM, -BRK, op0=OP.mult, op1=OP.add)
        en.vector.tensor_scalar(hi[:], sig[:], QNORM, BRK, op0=OP.mult, op1=OP.add)
        count_pass(lo, fl)
        count_pass(hi, fh)
        illinois_rounds(R_ITER1)

        # ---------- gate2 sweep 2: w_hi vs g_lo (accumulates onto sweep 1) ----
        for kt in range(32):
            w2k = wks.tile([128, HS], bf16, tag="wk2", name=f"w2h{kt}")
            en.sync.dma_start(w2k[:], w2h2_d[kt * 128:(kt + 1) * 128])
            gk = gkp.tile([128, T], bf16, tag="gkl", name=f"gkl{kt}")
            en.sync.dma_start(gk[:], kt_src(1, kt))
            for m in range(4):
                mslc = slice(m * 128, (m + 1) * 128)
                w_hi = w2k[:, mslc]
                en.tensor.matmul(g2ps[(m, 0)][:], w_hi, gk[:, n0],
                                 start=False, stop=(kt == 31),
                                 skip_group_check=True)
                en.tensor.matmul(g2ps[(m, 1)][:], w_hi, gk[:, n1],
                                 start=False, stop=(kt == 31),
                                 skip_group_check=True)

        # exact scores: colmax + A2A #2
        cmax = const.tile([128, 4, 8], f32, tag="cmax")
        a2a2_writers = []
        for m in range(4):
            scm = big.tile([128, T], f32, tag=f"sc{m}", name=f"sc{m}")
            for n in range(2):
                nslc = slice(n * 512, (n + 1) * 512)
                en.scalar.activation(scm[:, nslc], g2ps[(m, n)][:],
                                     AF.Identity, bias=b2_s[:, m:m + 1])
            en.vector.reduce_max(cmax[:, m, :],
                                 scm.rearrange("p (b s) -> p b s", s=S),
                                 axis=mybir.AxisListType.X)
            a2a2_writers.append(en.sync.dma_start(
                a2a_in[1][:, m * 128:(m + 1) * 128, :].rearrange("j p t -> p j t"),
                scm[:]))
        a2a2 = cc("AllToAll", OP.bypass, [a2a_in[1][:]], [a2a_out[1][:]],
                  waits=tuple(a2a2_writers))
        fill2 = en.sync.dma_start(
            scb[:], a2a_out[1].rearrange("r h t -> (r h t)").rearrange(
                "(p f) -> p f", p=128))
        tile.add_dep_helper(fill2.ins, a2a2.ins, reason="a2a2 -> fill")

        # ---------- main matmuls (overlap A2A #2 + search part 2) ----------
        hp_f = {m: big.tile([128, T], bf16, tag=f"hp{m}", name=f"hp{m}")
                for m in range(4)}
        hc_f = {m: big.tile([128, T], bf16, tag=f"hc{m}", name=f"hc{m}")
                for m in range(4)}
        for nh in range(2):
            nslc = slice(nh * 512, (nh + 1) * 512)
            mps = {(u, m): ps.tile([128, 512], f32, tag="ps",
                                   name=f"up_{nh}_{u}_{m}")
                   for u in range(2) for m in range(4)}
            for k in range(8):
                for u in range(2):
                    for m in range(4):
                        mslc = slice(m * 128, (m + 1) * 128)
                        en.tensor.matmul(mps[(u, m)][:], up_s[u][:, k, mslc],
                                         x_s[:, k, 0, nslc],
                                         start=(k == 0), stop=(k == 7))
            for u, bias_t, dst in ((0, bp_s, hp_f), (1, bc_s, hc_f)):
                for m in range(4):
                    en.scalar.activation(dst[m][:, nslc], mps[(u, m)][:],
                                         AF.Identity, bias=bias_t[:, m:m + 1])

        # ---------- search part 2 (exact scores, tight bracket) ----------
        part2 = en.vector.tensor_scalar(s1[:], lo[:], 1.0, None, op0=OP.mult)
        tile.add_dep_helper(part2.ins, fill2.ins, reason="exact scb ready")
        en.vector.tensor_scalar(lo[:], s1[:], 1.0, -BRK2, op0=OP.mult, op1=OP.add)
        en.vector.tensor_scalar(hi[:], s1[:], 1.0, BRK2, op0=OP.mult, op1=OP.add)
        count_pass(lo, fl)
        count_pass(hi, fh)
        illinois_rounds(R_ITER2)

        # broadcast my lo to an 8-wide row and AllGather all thresholds
        ones8 = const.tile([1, 8], f32, tag="ones8")
        en.vector.memset(ones8[:], 1.0)
        tsb = const.tile([1, 8], f32, tag="tsb")
        en.vector.tensor_scalar(tsb[:], ones8[:], lo[0:1, :], None, op0=OP.mult)
        t_w = en.sync.dma_start(t_ag_in[:], tsb[:])
        cc("AllGather", OP.bypass, [t_ag_in[:]], [t_ag_out[:]], waits=(t_w,))
        t_all = const.tile([1, 8], f32, tag="t_all")
        en.sync.dma_start(t_all[:], t_ag_out.rearrange(
            "(r k) -> r k", k=8)[:, 0:1].rearrange("r one -> one r"))
        t_bc = const.tile([128, 8], f32, tag="t_bc")
        en.gpsimd.partition_broadcast(t_bc[:], t_all[:], channels=128)

        # ---------- select + gelu ----------
        sel = const.tile([128, 4, 8], f32, tag="sel")
        for m in range(4):
            en.vector.tensor_tensor(sel[:, m, :], cmax[:, m, :], t_bc[:],
                                    op=OP.is_gt)
        a_sb = gact.tile([128, 4, T], bf16, tag="asb", name="a_sb")
        for m in range(4):
            dm = big.tile([128, T], bf16, tag="dm", name=f"d{m}")
            en.vector.tensor_sub(dm[:], hp_f[m][:], hc_f[m][:])
            hsel = hc_f[m]
            for b in range(B):
                bs = slice(b * S, (b + 1) * S)
                en.vector.scalar_tensor_tensor(
                    hsel[:, bs], dm[:, bs], sel[:, m, b:b + 1], hsel[:, bs],
                    op0=OP.mult, op1=OP.add)
            en.scalar.activation(a_sb[:, m], hsel[:], AF.Gelu)

        # ---------- down: partial_out[t, d] = act_shard.T @ dw_shard ----------
        for n in range(2):
            nslc = slice(n * 512, (n + 1) * 512)
            pts = {mt: ps.tile([128, 512], f32, tag="ps", name=f"o_{mt}_{n}")
                   for mt in range(B)}
            for k in range(4):
                for mt in range(B):
                    mslc = slice(mt * 128, (mt + 1) * 128)
                    en.tensor.matmul(pts[mt][:], a_sb[:, k, mslc],
                                     dw_s[:, k, nslc],
                                     start=(k == 0), stop=(k == 3))
            rs_writers = []
            for mt in range(B):
                osb = outp.tile([128, 512], bf16, tag="osb", name=f"osb{mt}_{n}")
                en.vector.tensor_tensor(osb[:], pts[mt][:], dbias_s[:, nslc],
                                        op=OP.add)
                rs_writers.append(en.sync.dma_start(rs_in[n][mt], osb[:]))
            cc("ReduceScatter", OP.add, [rs_in[n][:]], [rs_out[n][:]],
               waits=tuple(rs_writers))
            rsb = outp.tile([128, 512], bf16, tag="rsb", name=f"rsb{n}")
            en.sync.dma_start(rsb[:], rs_out[n][:])
            of = outp.tile([128, 512], f32, tag="of", name=f"of{n}")
            en.vector.tensor_copy(of[:], rsb[:])
            en.sync.dma_start(out_d[:, nslc], of[:])

    nc.compile()
    return nc


def _split_hl_pm(a, kt):
    """fp32 [K, M] lhsT -> partition-major stacked bf16 (hi, lo):
    [128, kt, 2, M] where K = kt*128."""
    hi = a.astype(BF16)
    lo = (a.astype(np.float64) - hi.astype(np.float64)).astype(BF16)
    st = np.stack([hi, lo], axis=1)            # [K, 2, M]
    M = a.shape[1]
    return np.ascontiguousarray(
        st.reshape(kt, 128, 2, M).transpose(1, 0, 2, 3))


_NC_CACHE = None


def _prep_in_maps(x, w1, b1, w2, b2, upw, upb, ucw, ucb, dw, db):
    xt = np.ascontiguousarray(x.reshape(T, D).T)     # [D, T]
    xhl = _split_hl_pm(xt, 8)                        # [128, 8, 2, T]
    # gate2 K-tile order follows the 3-chunk AllGather:
    # kt<8: rank kt, m=0; kt<16: rank kt-8, m=1; else rank (kt-16)//2, m=2+kt%2
    base = np.empty(32, np.int64)
    for kt in range(32):
        if kt < 8:
            base[kt] = kt * HS
        elif kt < 16:
            base[kt] = (kt - 8) * HS + 128
        else:
            idx = kt - 16
            base[kt] = (idx // 2) * HS + (2 + idx % 2) * 128
    w2_perm = (base[:, None] + np.arange(128)[None, :]).reshape(-1)

    def split_hl(a):
        hi = a.astype(BF16)
        lo = (a.astype(np.float64) - hi.astype(np.float64)).astype(BF16)
        return np.ascontiguousarray(np.stack([hi, lo], axis=1))

    def pm_bf(a, kt):
        """fp32 [K, M] -> [128, kt, M] partition-major bf16."""
        M = a.shape[1]
        return np.ascontiguousarray(
            a.reshape(kt, 128, M).transpose(1, 0, 2).astype(BF16))

    in_maps = []
    for c in range(NCORES):
        sh = slice(c * HS, (c + 1) * HS)
        dbias = np.tile(db[None, :], (128, 1)) if c == 0 else np.zeros((128, D), F32)
        in_maps.append({
            "xhl": xhl,
            "w1hl": _split_hl_pm(np.ascontiguousarray(w1[sh].T), 8),
            "w2hl": split_hl(np.ascontiguousarray(w2[sh].T[w2_perm])),
            "w2h2": np.ascontiguousarray(w2[sh].T.astype(BF16)),
            "uppb": pm_bf(np.ascontiguousarray(upw[sh].T), 8),
            "upcb": pm_bf(np.ascontiguousarray(ucw[sh].T), 8),
            "dwb": pm_bf(np.ascontiguousarray(dw[:, sh].T), 4),
            "b1s": np.ascontiguousarray(b1[sh].reshape(4, 128).T),
            "b2s": np.ascontiguousarray(b2[sh].reshape(4, 128).T),
            "bps": np.ascontiguousarray(upb[sh].reshape(4, 128).T),
            "bcs": np.ascontiguousarray(ucb[sh].reshape(4, 128).T),
            "dbias": np.ascontiguousarray(dbias.astype(F32)),
        })
    return in_maps


def kernel_in_maps(**inputs):
    names = ["inputs", "gate_w1", "gate_b1", "gate_w2", "gate_b2",
             "up_prev_w", "up_prev_b", "up_curr_w", "up_curr_b",
             "down_w", "down_b"]
    vals = [np.asarray(inputs[n], F32) for n in names]
    return _prep_in_maps(*vals)


def kernel(**inputs):
    global _NC_CACHE
    if _NC_CACHE is None:
        _NC_CACHE = _build()
    nc = _NC_CACHE
    in_maps = kernel_in_maps(**inputs)
    res = run_bass_kernel_spmd(nc, in_maps, core_ids=list(range(NCORES)))
    out = np.stack([res.results[c]["out"] for c in range(NCORES)], axis=0)
    return np.ascontiguousarray(out.astype(F32))


# revision 12
# speedup vs baseline: 1.0460x; 1.0460x over previous
"""Trainium2 Bass kernel for nn_PraxisScatter (moe_routing).

Strategy (8 NeuronCores, tensor-parallel over the hidden dim H=4096):
  - Each core owns a 512-row shard of H for gate1/gate2/up/down weights.
  - gate1: gT = relu(w1 @ xT + b1) on the shard, 3-term bf16 hi/lo matmuls
    (the top-k selection needs exact-grade scores).
  - g is AllGathered partition-major in three chunks (hi m01, hi m23, lo)
    so every gather-output load is one contiguous run per partition.
  - gate2 sweep 1 accumulates the two g_hi terms (w_hi+w_lo vs g_hi);
    the partial scores are AllToAll'd and the threshold search runs on
    them (sigma bracket + 9 Illinois rounds) WHILE sweep 2 (w_hi vs
    g_lo) finishes the exact scores on TensorE.
  - Exact scores are AllToAll'd; 8 more Illinois rounds in a +-4e-4
    bracket land in the exact count==K plateau (counts split: vector
    is_gt half, scalar Sign half, in parallel; reduce on GpSimd).
  - main: h_prev/h_curr = up @ xT single-term bf16; select in bf16;
    exact Gelu; down: single-term bf16 partials, bf16 ReduceScatter in
    two D-halves.
  - Data movement is descriptor-lean: DMAs are shaped so each transfer
    is one (or few) contiguous runs per partition -- the DMA dispatch
    sequencer, not any compute engine, was the prior bottleneck.
"""

import sys

try:
    import concourse  # noqa: F401
except ImportError:  # pragma: no cover
    sys.path.insert(0, "/opt/trn_rl_repo")

import contextlib

import ml_dtypes
import numpy as np

import concourse.bass as bass  # noqa: F401
import concourse.bass_isa as bass_isa
import concourse.mybir as mybir
import concourse.tile as tile
from concourse import bacc
from concourse.bass_utils import run_bass_kernel_spmd

BF16 = ml_dtypes.bfloat16
F32 = np.float32

NCORES = 8
B, S, D, H = 8, 128, 1024, 4096
T = B * S              # 1024 tokens
HS = H // NCORES       # 512 h rows per core
K_SEL = 256 * S        # 32768
QNORM = 1.5341         # Phi^-1(1 - K_SEL/(S*H))
BRK = 0.04             # part-1 bracket half-width around sigma estimate
BRK2 = 4e-4            # part-2 bracket half-width around partial threshold
R_ITER1 = 9            # Illinois rounds on partial scores (hidden)
R_ITER2 = 8            # Illinois rounds on exact scores

f32 = mybir.dt.float32
bf16 = mybir.dt.bfloat16
AF = mybir.ActivationFunctionType
OP = mybir.AluOpType


def _build():
    nc = bacc.Bacc("TRN2", target_bir_lowering=False, debug=False,
                   num_devices=NCORES)

    # ---- per-core DRAM parameters (host pre-transposed, partition-major) ----
    xhl_d = nc.dram_tensor("xhl", [128, 8, 2, T], bf16, kind="ExternalInput").ap()
    w1_d = nc.dram_tensor("w1hl", [128, 8, 2, HS], bf16, kind="ExternalInput").ap()
    # w2 sweep-1 stream: [p][kt][hl][m] with kt in AG-arrival order
    w2_d = nc.dram_tensor("w2hl", [128, 32, 2, HS], bf16, kind="ExternalInput").ap()
    # w2 sweep-2 stream (hi only): [p][kt][m] with kt in natural h order
    w2h2_d = nc.dram_tensor("w2h2", [128, 32, HS], bf16, kind="ExternalInput").ap()
    up_p_d = nc.dram_tensor("uppb", [128, 8, HS], bf16, kind="ExternalInput").ap()
    up_c_d = nc.dram_tensor("upcb", [128, 8, HS], bf16, kind="ExternalInput").ap()
    dw_d = nc.dram_tensor("dwb", [128, 4, D], bf16, kind="ExternalInput").ap()
    b1_d = nc.dram_tensor("b1s", [128, 4], f32, kind="ExternalInput").ap()
    b2_d = nc.dram_tensor("b2s", [128, 4], f32, kind="ExternalInput").ap()
    bp_d = nc.dram_tensor("bps", [128, 4], f32, kind="ExternalInput").ap()
    bc_d = nc.dram_tensor("bcs", [128, 4], f32, kind="ExternalInput").ap()
    dbias_d = nc.dram_tensor("dbias", [128, D], f32, kind="ExternalInput").ap()
    out_d = nc.dram_tensor("out", [S, D], f32, kind="ExternalOutput").ap()

    # ---- internal DRAM (collective bounce buffers) ----
    wu_in = nc.dram_tensor("wu_in", [8], f32).ap()
    wu_out = nc.dram_tensor("wu_out", [64], f32, addr_space="Shared").ap()
    wu2_out = nc.dram_tensor("wu2_out", [8], f32).ap()
    wu_rs_in = nc.dram_tensor("wu_rs_in", [64], f32).ap()
    wu3_out = nc.dram_tensor("wu3_out", [8], f32).ap()
    # partition-major g gathers: in [128, m, T]; out [rank, 128, m, T]
    g_hiA_in = nc.dram_tensor("g_hiA_in", [128, 2, T], bf16).ap()
    g_hiB_in = nc.dram_tensor("g_hiB_in", [128, 2, T], bf16).ap()
    g_lo_in = nc.dram_tensor("g_lo_in", [128, 4, T], bf16).ap()
    g_hiA_out = nc.dram_tensor("g_hiA_out", [NCORES, 128, 2, T], bf16,
                               addr_space="Shared").ap()
    g_hiB_out = nc.dram_tensor("g_hiB_out", [NCORES, 128, 2, T], bf16,
                               addr_space="Shared").ap()
    g_lo_out = nc.dram_tensor("g_lo_out", [NCORES, 128, 4, T], bf16,
                              addr_space="Shared").ap()
    # A2A chunk j content is p-major [128, m, t] so both sides are contiguous
    a2a_in = [nc.dram_tensor(f"a2a_in{i}", [NCORES, 128, 4, S], f32).ap()
              for i in range(2)]
    a2a_out = [nc.dram_tensor(f"a2a_out{i}", [NCORES, 128, 4, S], f32).ap()
               for i in range(2)]
    t_ag_in = nc.dram_tensor("t_ag_in", [8], f32).ap()
    t_ag_out = nc.dram_tensor("t_ag_out", [64], f32, addr_space="Shared").ap()
    rs_in = [nc.dram_tensor(f"rs_in{q}", [B, S, 512], bf16).ap() for q in range(2)]
    rs_out = [nc.dram_tensor(f"rs_out{q}", [S, 512], bf16).ap() for q in range(2)]

    rg = [list(range(NCORES))]

    with tile.TileContext(nc) as tc, contextlib.ExitStack() as ctx:
        en = tc.nc
        const = ctx.enter_context(tc.tile_pool(name="const", bufs=1))
        xp = ctx.enter_context(tc.tile_pool(name="xres", bufs=1))
        wks = ctx.enter_context(tc.tile_pool(name="wks", bufs=2))
        gkp = ctx.enter_context(tc.tile_pool(name="gkp", bufs=3))
        gact = ctx.enter_context(tc.tile_pool(name="gact", bufs=1))
        big = ctx.enter_context(tc.tile_pool(name="big", bufs=1))
        outp = ctx.enter_context(tc.tile_pool(name="outp", bufs=2))
        ps = ctx.enter_context(tc.tile_pool(name="ps", bufs=8, space="PSUM"))

        _cc_prev = [None]

        def cc(kind, op, ins, outs, waits=()):
            """Issue a collective: explicitly depend on every DMA that wrote
            the input buffer, and chain collectives so every core issues
            them in one fixed order."""
            h = en.gpsimd.collective_compute(kind, op, ins=ins, outs=outs,
                                             replica_groups=rg)
            for w in waits:
                tile.add_dep_helper(h.ins, w.ins,
                                    reason="collective input writer")
            if _cc_prev[0] is not None:
                tile.add_dep_helper(h.ins, _cc_prev[0].ins,
                                    reason="collective issue-order chain")
            _cc_prev[0] = h
            return h

        # warmup: absorb the CC engine's startup cost early
        cc("AllGather", OP.bypass, [wu_in[:]], [wu_out[:]])
        cc("AllToAll", OP.bypass, [wu_in[:]], [wu2_out[:]])
        cc("ReduceScatter", OP.add, [wu_rs_in[:]], [wu3_out[:]])

        def mm3_pair(p0, p1, lhsT_tile, rhs_tile, mslc, first, last):
            """3-term hi/lo accumulation into the (n0, n1) psum pair, ordered
            so consecutive matmuls share the stationary operand."""
            w_hi, w_lo = lhsT_tile[:, 0, mslc], lhsT_tile[:, 1, mslc]
            n0, n1 = slice(0, 512), slice(512, 1024)
            en.tensor.matmul(p0[:], w_hi, rhs_tile[:, 0, n0], start=first, stop=False)
            en.tensor.matmul(p1[:], w_hi, rhs_tile[:, 0, n1], start=first, stop=False)
            en.tensor.matmul(p0[:], w_hi, rhs_tile[:, 1, n0], start=False, stop=False)
            en.tensor.matmul(p1[:], w_hi, rhs_tile[:, 1, n1], start=False, stop=False)
            en.tensor.matmul(p0[:], w_lo, rhs_tile[:, 0, n0], start=False, stop=last)
            en.tensor.matmul(p1[:], w_lo, rhs_tile[:, 0, n1], start=False, stop=last)

        # ---------- resident loads ----------
        x_s = xp.tile([128, 8, 2, T], bf16, tag="x")
        w1r = xp.tile([128, 8, 2, HS], bf16, tag="w1r")
        for k in range(0, 8, 2):
            en.sync.dma_start(x_s[:, k:k + 2], xhl_d[:, k:k + 2])
        en.sync.dma_start(w1r[:], w1_d[:])
        b1_s = const.tile([128, 4], f32, tag="b1")
        en.sync.dma_start(b1_s[:], b1_d[:])
        b2_s = const.tile([128, 4], f32, tag="b2")
        en.sync.dma_start(b2_s[:], b2_d[:])
        bp_s = const.tile([128, 4], f32, tag="bp")
        en.sync.dma_start(bp_s[:], bp_d[:])
        bc_s = const.tile([128, 4], f32, tag="bc")
        en.sync.dma_start(bc_s[:], bc_d[:])

        # ---------- gate1: gT = relu(w1s @ xT + b1) [HS, T] ----------
        # g_sb layout [p, hl, m, T]: hi/lo halves are contiguous slabs so the
        # three gather-input writes are single-run-per-partition.
        g_sb = gact.tile([128, 2, 4, T], bf16, tag="gact", name="g_sb")
        for m in range(4):
            mslc = slice(m * 128, (m + 1) * 128)
            pts = {n: ps.tile([128, 512], f32, tag="ps", name=f"g1_{m}_{n}")
                   for n in range(2)}
            for k in range(8):
                mm3_pair(pts[0], pts[1], w1r[:, k], x_s[:, k], mslc,
                         first=(k == 0), last=(k == 7))
            gf = big.tile([128, T], f32, tag="gf", name=f"gf{m}")
            for n in range(2):
                nslc = slice(n * 512, (n + 1) * 512)
                en.scalar.activation(gf[:, nslc], pts[n][:],
                                     AF.Relu, bias=b1_s[:, m:m + 1])
            en.vector.tensor_copy(g_sb[:, 0, m, :], gf[:])
            en.vector.tensor_sub(g_sb[:, 1, m, :], gf[:], g_sb[:, 0, m, :])
            if m == 1:
                wA = en.sync.dma_start(g_hiA_in[:], g_sb[:, 0, 0:2, :])
                cc("AllGather", OP.bypass, [g_hiA_in[:]], [g_hiA_out[:]],
                   waits=(wA,))
            if m == 3:
                wB = en.sync.dma_start(g_hiB_in[:], g_sb[:, 0, 2:4, :])
                cc("AllGather", OP.bypass, [g_hiB_in[:]], [g_hiB_out[:]],
                   waits=(wB,))
                wL = en.sync.dma_start(g_lo_in[:], g_sb[:, 1])
                cc("AllGather", OP.bypass, [g_lo_in[:]], [g_lo_out[:]],
                   waits=(wL,))

        # ---------- gate2 sweep 1: (w_hi + w_lo) vs g_hi ----------
        # kt order: (rank, m01) over hiA then (rank, m23) over hiB; the host
        # lays w2hl out to match. Each rank's gather block loads as one
        # contiguous [128, 2, T] run per partition.
        g2ps = {(m, n): ps.tile([128, 512], f32, tag="ps", name=f"g2_{m}_{n}")
                for m in range(4) for n in range(2)}
        n0, n1 = slice(0, 512), slice(512, 1024)
        w2t = {}
        for c in range(8):
            w2t[c] = wks.tile([128, 4, 2, HS], bf16, tag="wk", name=f"w2c{c}")
            en.sync.dma_start(w2t[c][:], w2_d[:, c * 4:(c + 1) * 4])
        for ph, src_out in ((0, g_hiA_out), (1, g_hiB_out)):
            for r in range(NCORES):
                gk = gkp.tile([128, 2, T], bf16, tag="gks", name=f"gk{ph}_{r}")
                en.sync.dma_start(gk[:], src_out[r])
                for mm in range(2):
                    kt = ph * 16 + r * 2 + mm
                    w2k = w2t[kt // 4][:, kt % 4]
                    last = kt == 31
                    for m in range(4):
                        mslc = slice(m * 128, (m + 1) * 128)
                        w_hi, w_lo = w2k[:, 0, mslc], w2k[:, 1, mslc]
                        p0, p1 = g2ps[(m, 0)], g2ps[(m, 1)]
                        first = kt == 0
                        en.tensor.matmul(p0[:], w_hi, gk[:, mm, n0],
                                         start=first, stop=False)
                        en.tensor.matmul(p1[:], w_hi, gk[:, mm, n1],
                                         start=first, stop=False)
                        en.tensor.matmul(p0[:], w_lo, gk[:, mm, n0],
                                         start=False, stop=last)
                        en.tensor.matmul(p1[:], w_lo, gk[:, mm, n1],
                                         start=False, stop=last)

        # partial scores -> A2A #1 (j-major score tile; 8 clean chunk writes)
        sc_all = big.tile([128, 8, 4, S], f32, tag="scall", name="sc_all")
        for m in range(4):
            for n in range(2):
                en.scalar.activation(sc_all[:, 4 * n:4 * (n + 1), m, :],
                                     g2ps[(m, n)][:],
                                     AF.Identity, bias=b2_s[:, m:m + 1])
        a2a1_writers = [en.sync.dma_start(a2a_in[0][j], sc_all[:, j])
                        for j in range(NCORES)]
        cc("AllToAll", OP.bypass, [a2a_in[0][:]], [a2a_out[0][:]],
           waits=tuple(a2a1_writers))

        scb = big.tile([128, H], f32, tag="scb", name="scb")
        fill1 = en.sync.dma_start(
            scb[:], a2a_out[0].rearrange("r p m t -> (r p m t)").rearrange(
                "(p f) -> p f", p=128))
        tile.add_dep_helper(fill1.ins, _cc_prev[0].ins, reason="a2a1 -> fill")
        cmpb = big.tile([128, H], bf16, tag="cmpb", name="cmpb")
        up_s = {0: xp.tile([128, 8, HS], bf16, tag="upp", name="up_s0"),
                1: xp.tile([128, 8, HS], bf16, tag="upc", name="up_s1")}
        for u, src_d in ((0, up_p_d), (1, up_c_d)):
            en.sync.dma_start(up_s[u][:], src_d[:])
        dw_s = xp.tile([128, 4, D], bf16, tag="dw")
        en.sync.dma_start(dw_s[:], dw_d[:])
        dbias_s = const.tile([128, D], f32, tag="dbias")
        en.sync.dma_start(dbias_s[:], dbias_d[:])

        def sv(tag):
            return const.tile([128, 1], f32, tag=tag, name=tag)

        lo, hi, fl, fh = sv("lo"), sv("hi"), sv("fl"), sv("fh")
        tt, acc, acs, cnt = sv("tt"), sv("acc"), sv("acs"), sv("cnt")
        s1, s2, s3 = sv("s1"), sv("s2"), sv("s3")
        gt, ng, sig = sv("gt"), sv("ng"), sv("sig")
        HALF = H // 2
        CNT_OFF = float(K_SEL) - 0.5 - 128 * (HALF // 2)

        def count_pass(t_tile, f_out):
            """f_out = count(scb > t) - (K_SEL - 0.5), replicated on lanes.

            Vector engine counts the first half (is_gt), scalar engine the
            second half (Sign(t - s) summed), in parallel.
            """
            en.vector.tensor_scalar(cmpb[:, :HALF], scb[:, :HALF], t_tile[:],
                                    0.0, op0=OP.is_gt, op1=OP.add,
                                    accum_out=acc[:])
            en.scalar.activation(cmpb[:, HALF:], scb[:, HALF:], AF.Sign,
                                 bias=t_tile[:], scale=-1.0, accum_out=acs[:])
            en.vector.scalar_tensor_tensor(s3[:], acs[:], -0.5, acc[:],
                                           op0=OP.mult, op1=OP.add)
            en.gpsimd.partition_all_reduce(cnt[:], s3[:], channels=128,
                                           reduce_op=bass_isa.ReduceOp.add)
            en.vector.tensor_scalar(f_out[:], cnt[:], CNT_OFF, None,
                                    op0=OP.subtract)

        def illinois_rounds(n_rounds):
            for r in range(n_rounds):
                if (r + 1) % 4 == 0:
                    en.vector.tensor_tensor(s2[:], lo[:], hi[:], op=OP.add)
                    en.vector.tensor_scalar(tt[:], s2[:], 0.5, None, op0=OP.mult)
                else:
                    en.vector.tensor_tensor(s1[:], fl[:], fh[:], op=OP.subtract)
                    en.vector.reciprocal(s2[:], s1[:])
                    en.vector.tensor_tensor(s3[:], fl[:], s2[:], op=OP.mult)
                    en.vector.tensor_scalar(s3[:], s3[:], 0.02, 0.98,
                                            op0=OP.max, op1=OP.min)
                    en.vector.tensor_tensor(s1[:], hi[:], lo[:], op=OP.subtract)
                    en.vector.scalar_tensor_tensor(tt[:], s1[:], s3[:], lo[:],
                                                   op0=OP.mult, op1=OP.add)
                count_pass(tt, s1)  # s1 = f_t
                en.vector.tensor_scalar(gt[:], s1[:], 0.0, None, op0=OP.is_ge)
                en.vector.tensor_scalar(ng[:], gt[:], 1.0, -1.0,
                                        op0=OP.subtract, op1=OP.mult)  # 1-gt
                en.vector.tensor_tensor(s2[:], tt[:], lo[:], op=OP.subtract)
                en.vector.scalar_tensor_tensor(lo[:], s2[:], gt[:], lo[:],
                                               op0=OP.mult, op1=OP.add)
                en.vector.tensor_tensor(s2[:], s1[:], fl[:], op=OP.subtract)
                en.vector.scalar_tensor_tensor(fl[:], s2[:], gt[:], fl[:],
                                               op0=OP.mult, op1=OP.add)
                en.vector.tensor_tensor(s2[:], tt[:], hi[:], op=OP.subtract)
                en.vector.scalar_tensor_tensor(hi[:], s2[:], ng[:], hi[:],
                                               op0=OP.mult, op1=OP.add)
                en.vector.tensor_tensor(s2[:], s1[:], fh[:], op=OP.subtract)
                en.vector.scalar_tensor_tensor(fh[:], s2[:], ng[:], fh[:],
                                               op0=OP.mult, op1=OP.add)

        # part 1 (on partial scores; hidden under gate2 sweep 2)
        ssq = en.scalar.activation(cmpb[:], scb[:], AF.Square, accum_out=acc[:])
        tile.add_dep_helper(ssq.ins, fill1.ins, reason="scb fill barrier")
        en.gpsimd.partition_all_reduce(cnt[:], acc[:], channels=128,
                                       reduce_op=bass_isa.ReduceOp.add)
        en.scalar.activation(sig[:], cnt[:], AF.Sqrt, scale=1.0 / (S * H))
        en.vector.tensor_scalar(lo[:], sig[:], QNORM, -BRK, op0=OP.mult, op1=OP.add)
        en.vector.tensor_scalar(hi[:], sig[:], QNORM, BRK, op0=OP.mult, op1=OP.add)
        count_pass(lo, fl)
        count_pass(hi, fh)
        illinois_rounds(R_ITER1)

        # ---------- gate2 sweep 2: w_hi vs g_lo (accumulates onto sweep 1) ----
        # kt order is natural h order: (rank, m).
        w2t2 = {}
        for c in range(8):
            w2t2[c] = wks.tile([128, 4, HS], bf16, tag="wk2", name=f"w2h{c}")
            en.sync.dma_start(w2t2[c][:], w2h2_d[:, c * 4:(c + 1) * 4])
        for r in range(NCORES):
            for hh in range(2):
                gk = gkp.tile([128, 2, T], bf16, tag="gks", name=f"gl{r}_{hh}")
                en.sync.dma_start(gk[:], g_lo_out[r][:, 2 * hh:2 * hh + 2])
                for mm in range(2):
                    kt = r * 4 + hh * 2 + mm
                    w2k = w2t2[kt // 4][:, kt % 4]
                    last = kt == 31
                    for m in range(4):
                        mslc = slice(m * 128, (m + 1) * 128)
                        w_hi = w2k[:, mslc]
                        en.tensor.matmul(g2ps[(m, 0)][:], w_hi, gk[:, mm, n0],
                                         start=False, stop=last,
                                         skip_group_check=True)
                        en.tensor.matmul(g2ps[(m, 1)][:], w_hi, gk[:, mm, n1],
                                         start=False, stop=last,
                                         skip_group_check=True)

        # exact scores: overwrite sc_all, colmax, A2A #2
        cmax = const.tile([128, 8, 4], f32, tag="cmax")
        for m in range(4):
            for n in range(2):
                en.scalar.activation(sc_all[:, 4 * n:4 * (n + 1), m, :],
                                     g2ps[(m, n)][:],
                                     AF.Identity, bias=b2_s[:, m:m + 1])
        en.vector.reduce_max(cmax[:], sc_all[:], axis=mybir.AxisListType.X)
        a2a2_writers = [en.sync.dma_start(a2a_in[1][j], sc_all[:, j])
                        for j in range(NCORES)]
        a2a2 = cc("AllToAll", OP.bypass, [a2a_in[1][:]], [a2a_out[1][:]],
                  waits=tuple(a2a2_writers))
        fill2 = en.sync.dma_start(
            scb[:], a2a_out[1].rearrange("r p m t -> (r p m t)").rearrange(
                "(p f) -> p f", p=128))
        tile.add_dep_helper(fill2.ins, a2a2.ins, reason="a2a2 -> fill")

        # ---------- main matmuls (overlap A2A #2 + search part 2) ----------
        hp_f = {m: big.tile([128, T], bf16, tag=f"hp{m}", name=f"hp{m}")
                for m in range(4)}
        hc_f = {m: big.tile([128, T], bf16, tag=f"hc{m}", name=f"hc{m}")
                for m in range(4)}
        for nh in range(2):
            nslc = slice(nh * 512, (nh + 1) * 512)
            mps = {(u, m): ps.tile([128, 512], f32, tag="ps",
                                   name=f"up_{nh}_{u}_{m}")
                   for u in range(2) for m in range(4)}
            for k in range(8):
                for u in range(2):
                    for m in range(4):
                        mslc = slice(m * 128, (m + 1) * 128)
                        en.tensor.matmul(mps[(u, m)][:], up_s[u][:, k, mslc],
                                         x_s[:, k, 0, nslc],
                                         start=(k == 0), stop=(k == 7))
            for u, bias_t, dst in ((0, bp_s, hp_f), (1, bc_s, hc_f)):
                for m in range(4):
                    en.scalar.activation(dst[m][:, nslc], mps[(u, m)][:],
                                         AF.Identity, bias=bias_t[:, m:m + 1])

        # ---------- search part 2 (exact scores, tight bracket) ----------
        part2 = en.vector.tensor_scalar(s1[:], lo[:], 1.0, None, op0=OP.mult)
        tile.add_dep_helper(part2.ins, fill2.ins, reason="exact scb ready")
        en.vector.tensor_scalar(lo[:], s1[:], 1.0, -BRK2, op0=OP.mult, op1=OP.add)
        en.vector.tensor_scalar(hi[:], s1[:], 1.0, BRK2, op0=OP.mult, op1=OP.add)
        count_pass(lo, fl)
        count_pass(hi, fh)
        illinois_rounds(R_ITER2)

        # broadcast my lo to an 8-wide row and AllGather all thresholds
        ones8 = const.tile([1, 8], f32, tag="ones8")
        en.vector.memset(ones8[:], 1.0)
        tsb = const.tile([1, 8], f32, tag="tsb")
        en.vector.tensor_scalar(tsb[:], ones8[:], lo[0:1, :], None, op0=OP.mult)
        t_w = en.sync.dma_start(t_ag_in[:], tsb[:])
        cc("AllGather", OP.bypass, [t_ag_in[:]], [t_ag_out[:]], waits=(t_w,))
        t_all = const.tile([1, 8], f32, tag="t_all")
        en.sync.dma_start(t_all[:], t_ag_out.rearrange(
            "(r k) -> r k", k=8)[:, 0:1].rearrange("r one -> one r"))
        t_bc = const.tile([128, 8], f32, tag="t_bc")
        en.gpsimd.partition_broadcast(t_bc[:], t_all[:], channels=128)

        # ---------- select + gelu ----------
        sel = const.tile([128, 4, 8], f32, tag="sel")
        for m in range(4):
            en.vector.tensor_tensor(sel[:, m, :], cmax[:, :, m], t_bc[:],
                                    op=OP.is_gt)
        a_sb = gact.tile([128, 4, T], bf16, tag="asb", name="a_sb")
        for m in range(4):
            dm = big.tile([128, T], bf16, tag="dm", name=f"d{m}")
            en.vector.tensor_sub(dm[:], hp_f[m][:], hc_f[m][:])
            hsel = hc_f[m]
            for b in range(B):
                bs = slice(b * S, (b + 1) * S)
                en.vector.scalar_tensor_tensor(
                    hsel[:, bs], dm[:, bs], sel[:, m, b:b + 1], hsel[:, bs],
                    op0=OP.mult, op1=OP.add)
            en.scalar.activation(a_sb[:, m], hsel[:], AF.Gelu)

        # ---------- down: partial_out[t, d] = act_shard.T @ dw_shard ----------
        for n in range(2):
            nslc = slice(n * 512, (n + 1) * 512)
            pts = {mt: ps.tile([128, 512], f32, tag="ps", name=f"o_{mt}_{n}")
                   for mt in range(B)}
            for k in range(4):
                for mt in range(B):
                    mslc = slice(mt * 128, (mt + 1) * 128)
                    en.tensor.matmul(pts[mt][:], a_sb[:, k, mslc],
                                     dw_s[:, k, nslc],
                                     start=(k == 0), stop=(k == 3))
            rs_writers = []
            for mt in range(B):
                osb = outp.tile([128, 512], bf16, tag="osb", name=f"osb{mt}_{n}")
                en.vector.tensor_tensor(osb[:], pts[mt][:], dbias_s[:, nslc],
                                        op=OP.add)
                rs_writers.append(en.sync.dma_start(rs_in[n][mt], osb[:]))
            cc("ReduceScatter", OP.add, [rs_in[n][:]], [rs_out[n][:]],
               waits=tuple(rs_writers))
            rsb = outp.tile([128, 512], bf16, tag="rsb", name=f"rsb{n}")
            en.sync.dma_start(rsb[:], rs_out[n][:])
            of = outp.tile([128, 512], f32, tag="of", name=f"of{n}")
            en.vector.tensor_copy(of[:], rsb[:])
            en.sync.dma_start(out_d[:, nslc], of[:])

    nc.compile()
    return nc


def _split_hl_pm(a, kt):
    """fp32 [K, M] lhsT -> partition-major stacked bf16 (hi, lo):
    [128, kt, 2, M] where K = kt*128."""
    hi = a.astype(BF16)
    lo = (a.astype(np.float64) - hi.astype(np.float64)).astype(BF16)
    st = np.stack([hi, lo], axis=1)            # [K, 2, M]
    M = a.shape[1]
    return np.ascontiguousarray(
        st.reshape(kt, 128, 2, M).transpose(1, 0, 2, 3))


_NC_CACHE = None


def _prep_in_maps(x, w1, b1, w2, b2, upw, upb, ucw, ucb, dw, db):
    xt = np.ascontiguousarray(x.reshape(T, D).T)     # [D, T]
    xhl = _split_hl_pm(xt, 8)                        # [128, 8, 2, T]
    # sweep-1 kt order: (rank, m) with m in {0,1} over hiA then {2,3} over hiB
    base = np.empty(32, np.int64)
    for kt in range(32):
        ph, rm = kt // 16, kt % 16
        r, m = rm // 2, rm % 2 + 2 * ph
        base[kt] = r * HS + m * 128
    w2_perm = (base[:, None] + np.arange(128)[None, :]).reshape(-1)

    def pm_bf(a, kt):
        """fp32 [K, M] -> [128, kt, M] partition-major bf16."""
        M = a.shape[1]
        return np.ascontiguousarray(
            a.reshape(kt, 128, M).transpose(1, 0, 2).astype(BF16))

    in_maps = []
    for c in range(NCORES):
        sh = slice(c * HS, (c + 1) * HS)
        dbias = np.tile(db[None, :], (128, 1)) if c == 0 else np.zeros((128, D), F32)
        w2T = np.ascontiguousarray(w2[sh].T)         # [H, HS] fp32
        in_maps.append({
            "xhl": xhl,
            "w1hl": _split_hl_pm(np.ascontiguousarray(w1[sh].T), 8),
            "w2hl": _split_hl_pm(np.ascontiguousarray(w2T[w2_perm]), 32),
            "w2h2": pm_bf(w2T, 32),
            "uppb": pm_bf(np.ascontiguousarray(upw[sh].T), 8),
            "upcb": pm_bf(np.ascontiguousarray(ucw[sh].T), 8),
            "dwb": pm_bf(np.ascontiguousarray(dw[:, sh].T), 4),
            "b1s": np.ascontiguousarray(b1[sh].reshape(4, 128).T),
            "b2s": np.ascontiguousarray(b2[sh].reshape(4, 128).T),
            "bps": np.ascontiguousarray(upb[sh].reshape(4, 128).T),
            "bcs": np.ascontiguousarray(ucb[sh].reshape(4, 128).T),
            "dbias": np.ascontiguousarray(dbias.astype(F32)),
        })
    return in_maps


def kernel_in_maps(**inputs):
    names = ["inputs", "gate_w1", "gate_b1", "gate_w2", "gate_b2",
             "up_prev_w", "up_prev_b", "up_curr_w", "up_curr_b",
             "down_w", "down_b"]
    vals = [np.asarray(inputs[n], F32) for n in names]
    return _prep_in_maps(*vals)


def kernel(**inputs):
    global _NC_CACHE
    if _NC_CACHE is None:
        _NC_CACHE = _build()
    nc = _NC_CACHE
    in_maps = kernel_in_maps(**inputs)
    res = run_bass_kernel_spmd(nc, in_maps, core_ids=list(range(NCORES)))
    out = np.stack([res.results[c]["out"] for c in range(NCORES)], axis=0)
    return np.ascontiguousarray(out.astype(F32))
